# revision 2
# baseline (speedup 1.0000x reference)
"""Trainium2 Bass kernel for nn_Decoder (LSTM, B=128 T=512 H=1024 O=128).

Strategy: the T=512 recurrence is inherently sequential and one step's
recurrent matmul (h @ W_hh.T: 128x1024x4096) already saturates a single
NeuronCore's PE for ~9.5us, while any cross-core exchange of h costs an
8-core AllGather floor of ~5us + HBM bounces per step. Tensor-parallel
sharding therefore cannot beat replication, so every core runs the full
recurrence (weights and state replicated); the output is taken from core 0.

Per step (on each core):
  gates = [hT;x_t;1].T @ [W_hh.T; w_ih; b]   in bf16 on the PE,
          accumulated fp32 in PSUM, N=512 tiles, K tiled 8x128 (+K=2 aug).
  Gate columns are host-permuted per 128-wide H-chunk as [i|f|o|g] so one
  strided sigmoid covers i,f,o of a chunk pair and one tanh covers g.
  c (fp32) and h (bf16) updated on DVE; tanh/sigmoid on ACT;
  h chunks transposed back to lhsT layout [H,B] via the DMA xbar (2-byte).
"""

import os
import sys

sys.path.insert(0, "/opt/trn_rl_repo")
os.environ.setdefault("JAX_PLATFORMS", "")

from contextlib import ExitStack

import numpy as np
import ml_dtypes

import concourse.bass as bass
import concourse.mybir as mybir
import concourse.tile as tile
from concourse.bass import ds
from concourse.bass_utils import run_bass_kernel_spmd

B, T, H, O = 128, 512, 1024, 128
KC = H // 128          # 8 K-tiles of the contraction over H
NCH = H // 128         # 8 H-chunks of 128 hidden units
GW = 512               # gate columns per H-chunk: [i|f|o|g] x 128
BF16 = mybir.dt.bfloat16
F32 = mybir.dt.float32

_N_CORES = int(os.environ.get("KERNEL_N_CORES", "1"))
_UNROLL = int(os.environ.get("KERNEL_UNROLL", "4"))  # steps per For_i body (even)


# ---------------------------------------------------------------- drain patch
# walrus codegen limit: InstDrain on the SP engine accepts a single sync-wait
# command, but TileContext's exit drain aggregates one wait per outstanding
# logical processor onto one drain. Split them across a chain of drains.
def _apply_drain_patch():
    import concourse.tile as _tile
    from concourse.vector_clock import ScopedClock as _ScopedClock

    if getattr(_tile.TileContext, "_drain_patch_applied", False):
        return

    def _patched(self, tick_clock, wait_clock):
        drain_inst = self.nc.sync.drain()
        wait_clock.add_sem_waits(
            drain_inst.ins, _ScopedClock({None: tick_clock.global_clock})
        )
        si = drain_inst.ins.sync_info
        waits = list(si.on_wait) if si is not None and si.on_wait else []
        if len(waits) > 1:
            si.on_wait = waits[:1]
            for w in waits[1:]:
                extra = self.nc.sync.drain()
                extra.ins.sync_info = mybir.SyncInfo(on_wait=[w], on_update=[])
        self.nc.all_engine_barrier()
        assert self.sems is not None
        popped = self.nc._tile_sem_poison_stack.pop()
        assert popped is self._sem_poison
        self.nc.clear_and_free_semaphores(list(self.sems.allocated().values()))
        self.nc.all_engine_barrier()

    _tile.TileContext._drain_and_barrier = _patched
    _tile.TileContext._drain_patch_applied = True


# ----------------------------------------------------- wait-splitting post-pass
# This walrus build accepts at most 2 sync-wait commands on ordinary engine
# instructions and only 1 on SP/TPB_CTRL-class instructions (Drain, SP DMA
# triggers). Tile attaches up to ~4. Split the excess onto InstNoOp carriers
# inserted immediately before the offending instruction on the same engine.
_SP_LIKE = ("SP",)


def _wait_limit(inst):
    # empirically: TPB_CTRL (Drain) and S3S3D3_TT (TensorTensor) templates
    # accept a single sync-wait; play safe and allow one everywhere.
    return 1


def _split_excess_waits(nc):
    n_added = 0
    for f in nc.m.functions:
        for bb in f.blocks:
            insts = bb.instructions
            out = []
            changed = False
            for inst in insts:
                si = inst.sync_info
                waits = list(si.on_wait) if si is not None and si.on_wait else []
                lim = _wait_limit(inst)
                if len(waits) > lim:
                    keep = waits[len(waits) - lim :]
                    rest = waits[: len(waits) - lim]
                    nop_lim = 1
                    while rest:
                        chunk, rest = rest[:nop_lim], rest[nop_lim:]
                        nop = mybir.InstNoOp(
                            name=f"waitnop-{n_added}", ins=[], outs=[]
                        )
                        nop.engine = inst.engine
                        nop.sync_info = mybir.SyncInfo(on_wait=chunk, on_update=[])
                        out.append(nop)
                        n_added += 1
                    si.on_wait = keep
                    changed = True
                out.append(inst)
            if changed:
                bb.instructions = out
    return n_added


# ------------------------------------------------------------- program build
def build_program(t_steps=T, unroll=_UNROLL, debug_state=False, split_waits=True):
    _apply_drain_patch()
    assert t_steps % unroll == 0 and unroll % 2 == 0
    nc = bass.Bass("TRN2", debug=False)

    wt_d = nc.dram_tensor("wt", (H, 4 * H), BF16, kind="ExternalInput").ap()
    wihb_d = nc.dram_tensor("wihb", (2, 4 * H), BF16, kind="ExternalInput").ap()
    xa_d = nc.dram_tensor("xa", (2 * T, B), BF16, kind="ExternalInput").ap()
    ht0_d = nc.dram_tensor("ht0", (H, B), BF16, kind="ExternalInput").ap()
    c0_d = nc.dram_tensor("c0", (B, H), F32, kind="ExternalInput").ap()
    fcw_d = nc.dram_tensor("fcw", (128, H), BF16, kind="ExternalInput").ap()
    fca_d = nc.dram_tensor("fca", (2, 128), BF16, kind="ExternalInput").ap()
    id_d = nc.dram_tensor("ident", (128, 128), BF16, kind="ExternalInput").ap()
    out_d = nc.dram_tensor("out", (B, O), F32, kind="ExternalOutput").ap()
    if debug_state:
        ht_dbg_d = nc.dram_tensor(
            "ht_dbg", (NCH, 128, B), BF16, kind="ExternalOutput"
        ).ap()
        c_dbg_d = nc.dram_tensor("c_dbg", (B, H), F32, kind="ExternalOutput").ap()

    with tile.TileContext(nc) as tc:
        with ExitStack() as ctx:
            consts = ctx.enter_context(tc.tile_pool(name="consts", bufs=1))
            state = ctx.enter_context(tc.tile_pool(name="state", bufs=1))
            work = ctx.enter_context(tc.tile_pool(name="work", bufs=int(os.environ.get("WORK_BUFS", "3"))))
            xap = ctx.enter_context(tc.tile_pool(name="xap", bufs=int(os.environ.get("XA_BUFS", "4"))))
            psum = ctx.enter_context(tc.tile_pool(name="psum", bufs=3, space="PSUM"))
            fcp = ctx.enter_context(tc.tile_pool(name="fcp", bufs=1, space="PSUM"))
            ptp = ctx.enter_context(tc.tile_pool(name="ptp", bufs=1, space="PSUM"))


            # resident weights
            w_sb = []
            for k in range(KC):
                w_k = consts.tile([128, 4 * H], BF16, tag=f"w{k}", name=f"w{k}")
                nc.gpsimd.dma_start(out=w_k, in_=wt_d[k * 128 : (k + 1) * 128, :])
                w_sb.append(w_k)
            wihb = consts.tile([2, 4 * H], BF16, tag="wihb")
            nc.gpsimd.dma_start(out=wihb, in_=wihb_d)
            fcw = consts.tile([128, H], BF16, tag="fcw")
            nc.gpsimd.dma_start(out=fcw, in_=fcw_d)
            ident = consts.tile([128, 128], BF16, tag="ident")
            nc.gpsimd.dma_start(out=ident, in_=id_d)
            fcb_t = consts.tile([1, 128], BF16, tag="fcb_t")
            nc.gpsimd.dma_start(out=fcb_t, in_=fca_d[0:1, :])
            ones_t = consts.tile([1, 128], BF16, tag="ones_t")
            nc.gpsimd.dma_start(out=ones_t, in_=fca_d[1:2, :])

            # state: hT ping-pong chunk tiles, fp32 cell
            ht_a = [state.tile([128, B], BF16, tag=f"hta{k}", name=f"hta{k}") for k in range(NCH)]
            ht_b = [state.tile([128, B], BF16, tag=f"htb{k}", name=f"htb{k}") for k in range(NCH)]
            c_sb = state.tile([B, H], F32, tag="c")
            for k in range(NCH):
                nc.gpsimd.dma_start(
                    out=ht_a[k], in_=ht0_d[k * 128 : (k + 1) * 128, :]
                )
            nc.gpsimd.dma_start(out=c_sb, in_=c0_d)

            def step(iv_base, local_t, cur, nxt):
                """One LSTM step. iv_base: ScalarValue or int (loop index of the
                body start); local_t: python int offset within the body."""
                xa = xap.tile([2, B], BF16, tag="xa")
                # inside the For_i body only HWDGE DMAs are usable: the loop
                # reset block emits InstIncSwdgeSem for SWDGE queues, which
                # this walrus cannot encode ("ISA wrong length").
                if isinstance(iv_base, int):
                    off = 2 * (iv_base + local_t)
                    nc.sync.dma_start(out=xa, in_=xa_d[off : off + 2, :])
                else:
                    off = (iv_base + local_t) * 2
                    nc.sync.dma_start(out=xa, in_=xa_d[ds(off, 2), :])

                n_pairs = 3 if os.environ.get("TAIL_SINGLE", "0") == "1" else 4
                for p in range(n_pairs):  # pairs of H-chunks
                    ps = psum.tile([B, 2 * GW], F32, tag="gates", name=f"ps{p}")
                    for half in range(2):
                        cc = 2 * p + half
                        sl = ps[:, half * GW : (half + 1) * GW]
                        # K-order [0..5, aug, 6, 7]: defers the previous
                        # step's latest h-chunks by two MM slots, shrinking
                        # the step-boundary stall. Same PSUM group, so no
                        # tile-switch penalty.
                        for k in range(6):
                            nc.tensor.matmul(
                                sl,
                                lhsT=cur[k],
                                rhs=w_sb[k][:, cc * GW : (cc + 1) * GW],
                                start=(k == 0),
                                stop=False,
                            )
                        nc.tensor.matmul(
                            sl,
                            lhsT=xa,
                            rhs=wihb[:, cc * GW : (cc + 1) * GW],
                            start=False,
                            stop=False,
                        )
                        for k in (6, 7):
                            nc.tensor.matmul(
                                sl,
                                lhsT=cur[k],
                                rhs=w_sb[k][:, cc * GW : (cc + 1) * GW],
                                start=False,
                                stop=(k == KC - 1),
                            )
                    # eltwise; psum layout [i0 f0 o0 g0 i1 f1 o1 g1]
                    ps3 = ps.rearrange("p (c x) -> p c x", c=2)
                    sig = work.tile([B, 2, 384], BF16, tag="sig", name="sig")
                    nc.scalar.activation(
                        sig, ps3[:, :, 0:384], mybir.ActivationFunctionType.Sigmoid
                    )
                    tg = work.tile([B, 2, 128], BF16, tag="tg", name="tg")
                    nc.scalar.activation(
                        tg, ps3[:, :, 384:512], mybir.ActivationFunctionType.Tanh
                    )
                    sig_i = sig[:, :, 0:128]
                    sig_f = sig[:, :, 128:256]
                    sig_o = sig[:, :, 256:384]
                    c3 = c_sb[:, p * 256 : (p + 1) * 256].rearrange(
                        "p (c x) -> p c x", c=2
                    )
                    t1 = work.tile([B, 2, 128], F32, tag="t1", name="t1")
                    nc.vector.tensor_mul(t1, sig_f, c3)
                    t2 = work.tile([B, 2, 128], BF16, tag="t2", name="t2")
                    nc.vector.tensor_mul(t2, sig_i, tg)
                    nc.vector.tensor_add(c3, t1, t2)
                    tanc = work.tile([B, 2, 128], BF16, tag="tanc", name="tanc")
                    nc.scalar.activation(
                        tanc, c3, mybir.ActivationFunctionType.Tanh
                    )
                    hbf = work.tile([B, 2, 128], BF16, tag="hbf", name="hbf")
                    nc.vector.tensor_mul(hbf, sig_o, tanc)
                    for half in range(2):
                        if p >= 2:
                            # last pair is on the next step's critical path:
                            # PE transpose (~0.4us) beats the DMA xbar (~1.3us)
                            pt = ptp.tile([128, B], BF16, tag="pt", name="pt")
                            nc.tensor.transpose(pt, hbf[:, half, :], ident)
                            nc.vector.tensor_copy(nxt[2 * p + half], pt)
                        else:
                            nc.sync.dma_start_transpose(
                                nxt[2 * p + half], hbf[:, half, :]
                            )

                for cc in range(2 * n_pairs, NCH):  # tail chunks, single width
                    ps1 = psum.tile([B, GW], F32, tag="gates", name=f"ps1_{cc}")
                    for k in range(KC):
                        nc.tensor.matmul(
                            ps1, lhsT=cur[k],
                            rhs=w_sb[k][:, cc * GW : (cc + 1) * GW],
                            start=(k == 0), stop=False,
                        )
                    nc.tensor.matmul(
                        ps1, lhsT=xa, rhs=wihb[:, cc * GW : (cc + 1) * GW],
                        start=False, stop=True,
                    )
                    sig1 = work.tile([B, 384], BF16, tag="sig1", name="sig1")
                    nc.scalar.activation(
                        sig1, ps1[:, 0:384], mybir.ActivationFunctionType.Sigmoid
                    )
                    tg1 = work.tile([B, 128], BF16, tag="tg1", name="tg1")
                    nc.scalar.activation(
                        tg1, ps1[:, 384:512], mybir.ActivationFunctionType.Tanh
                    )
                    c1 = c_sb[:, cc * 128 : (cc + 1) * 128]
                    t1s = work.tile([B, 128], F32, tag="t1s", name="t1s")
                    nc.vector.tensor_mul(t1s, sig1[:, 128:256], c1)
                    t2s = work.tile([B, 128], BF16, tag="t2s", name="t2s")
                    nc.vector.tensor_mul(t2s, sig1[:, 0:128], tg1)
                    nc.vector.tensor_add(c1, t1s, t2s)
                    tanc1 = work.tile([B, 128], BF16, tag="tanc1", name="tanc1")
                    nc.scalar.activation(
                        tanc1, c1, mybir.ActivationFunctionType.Tanh
                    )
                    hbf1 = work.tile([B, 128], BF16, tag="hbf1", name="hbf1")
                    nc.vector.tensor_mul(hbf1, sig1[:, 256:384], tanc1)
                    pt1 = ptp.tile([128, B], BF16, tag="pt", name="pt1")
                    nc.tensor.transpose(pt1, hbf1, ident)
                    nc.vector.tensor_copy(nxt[cc], pt1)

            if t_steps == 0:
                pass
            elif t_steps <= unroll:
                repeat_u = int(os.environ.get("KERNEL_REPEAT", "1"))

                def unrolled_body():
                    for t in range(t_steps):
                        cur, nxt = (ht_a, ht_b) if t % 2 == 0 else (ht_b, ht_a)
                        step(0, t, cur, nxt)

                if repeat_u == 1:
                    unrolled_body()
                else:
                    with tc.For_i(0, repeat_u, 1):
                        unrolled_body()
            else:
                hints = tuple(mybir.ALL_ENGINES) if os.environ.get("HINTS", "0") == "1" else ()
                repeat = int(os.environ.get("KERNEL_REPEAT", "1"))

                def inner_loop():
                    with tc.For_i(0, t_steps, unroll, hint_engines=hints) as iv:
                        for j in range(unroll):
                            cur, nxt = (ht_a, ht_b) if j % 2 == 0 else (ht_b, ht_a)
                            step(iv, j, cur, nxt)

                if repeat == 1:
                    inner_loop()
                else:  # timing amplification only: state re-evolves from t=0 xs
                    with tc.For_i(0, repeat, 1):
                        inner_loop()

            ht_fin = ht_a if t_steps % 2 == 0 else ht_b

            # final FC: out = h_T @ fc_W.T + fc_b
            fc_ps = fcp.tile([B, O], F32, tag="fc", name="fcps")
            nc.tensor.matmul(
                fc_ps, lhsT=ones_t, rhs=fcb_t, start=True, stop=False
            )
            for k in range(KC):
                nc.tensor.matmul(
                    fc_ps,
                    lhsT=ht_fin[k],
                    rhs=fcw[:, k * 128 : (k + 1) * 128],
                    start=False,
                    stop=(k == KC - 1),
                )
            out_sb = work.tile([B, O], F32, tag="out_sb")
            nc.vector.tensor_copy(out_sb, fc_ps)
            nc.gpsimd.dma_start(out=out_d, in_=out_sb)

            if debug_state:
                for k in range(NCH):
                    nc.gpsimd.dma_start(out=ht_dbg_d[k], in_=ht_fin[k])
                nc.gpsimd.dma_start(out=c_dbg_d, in_=c_sb)

    if split_waits:  # required for walrus codegen; CoreSim chokes on it
        _split_excess_waits(nc)
    return nc


# ------------------------------------------------------------------ host prep
def _prep_inputs(y_hist, W_ih, W_hh, b_ih, b_hh, fc_W, fc_b, h0, c0):
    f32 = np.float32
    bf16 = ml_dtypes.bfloat16
    # per-chunk gate permutation of the 4H rows: [i_c | f_c | o_c | g_c]
    # reference gate order in rows of W_hh is (i, f, g, o) blocks of H
    perm = np.concatenate(
        [
            g * H + c * 128 + np.arange(128)
            for c in range(NCH)
            for g in (0, 1, 3, 2)
        ]
    )
    wt = np.ascontiguousarray(W_hh[perm, :].T).astype(bf16)          # (H, 4H)
    wihb = np.stack([W_ih[:, 0][perm], (b_ih + b_hh)[perm]]).astype(bf16)
    xa = np.empty((2 * T, B), f32)
    xa[0::2] = y_hist.T                                               # x_t rows
    xa[1::2] = 1.0                                                    # ones rows
    xa = xa.astype(bf16)
    ht0 = np.ascontiguousarray(h0.T).astype(bf16)                     # (H, B)
    fcw = np.ascontiguousarray(fc_W.T).astype(bf16)                  # (H, O)
    # device layout for fcw tile: (128, H) with chunk k at cols [128k:128k+128)
    fcw_tile = fcw.reshape(KC, 128, O).transpose(1, 0, 2).reshape(128, H)
    fca = np.stack([fc_b, np.ones(O, f32)]).astype(bf16)              # rhs, ones
    ident = np.eye(128, dtype=f32).astype(bf16)
    return {
        "ident": np.asarray(ident),
        "wt": np.asarray(wt),
        "wihb": np.asarray(wihb),
        "xa": np.asarray(xa),
        "ht0": np.asarray(ht0),
        "c0": c0.astype(f32),
        "fcw": np.asarray(fcw_tile),
        "fca": np.asarray(fca),
    }


_CACHE = {}


def _make_runner(nc):
    """Single-core reusable jitted executor (mirrors bass2jax.run_bass_via_pjrt
    but caches the jitted body so repeated kernel() calls skip retracing)."""
    import jax
    from concourse import bass2jax

    bass2jax.install_neuronx_cc_hook()
    partition_name = nc.partition_id_tensor.name if nc.partition_id_tensor else None
    in_names, out_names, out_avals, zero_outs = [], [], [], []
    for alloc in nc.m.functions[0].allocations:
        if not isinstance(alloc, mybir.MemoryLocationSet):
            continue
        name = alloc.memorylocations[0].name
        if alloc.kind == "ExternalInput":
            if name != partition_name:
                in_names.append(name)
        elif alloc.kind == "ExternalOutput":
            shape = tuple(alloc.tensor_shape)
            dtype = mybir.dt.np(alloc.dtype)
            out_names.append(name)
            out_avals.append(jax.core.ShapedArray(shape, dtype))
            zero_outs.append(np.zeros(shape, dtype))
    all_in = list(in_names) + list(out_names)
    if partition_name is not None:
        all_in.append(partition_name)

    def _body(*args):
        operands = list(args)
        if partition_name is not None:
            operands.append(bass2jax.partition_id_tensor())
        return tuple(
            bass2jax._bass_exec_p.bind(
                *operands,
                out_avals=tuple(out_avals),
                in_names=tuple(all_in),
                out_names=tuple(out_names),
                lowering_input_output_aliases=(),
                sim_require_finite=True,
                sim_require_nnan=True,
                nc=nc,
            )
        )

    f = jax.jit(_body, keep_unused=True)
    return f, in_names, out_names, zero_outs


def kernel(y_hist, W_ih, W_hh, b_ih, b_hh, fc_W, fc_b, h0, c0, **kw):
    dev_in = _prep_inputs(
        np.asarray(y_hist, np.float32),
        np.asarray(W_ih, np.float32),
        np.asarray(W_hh, np.float32),
        np.asarray(b_ih, np.float32),
        np.asarray(b_hh, np.float32),
        np.asarray(fc_W, np.float32),
        np.asarray(fc_b, np.float32),
        np.asarray(h0, np.float32),
        np.asarray(c0, np.float32),
    )
    if _N_CORES != 1:
        if "nc" not in _CACHE:
            _CACHE["nc"] = build_program()
        res = run_bass_kernel_spmd(
            _CACHE["nc"],
            [dict(dev_in) for _ in range(_N_CORES)],
            core_ids=list(range(_N_CORES)),
        )
        return np.asarray(res.results[0]["out"], np.float32)
    if "runner" not in _CACHE:
        nc = build_program()
        _CACHE["runner"] = _make_runner(nc)
    f, in_names, out_names, zero_outs = _CACHE["runner"]
    args = [np.asarray(dev_in[n]) for n in in_names] + zero_outs
    outs = f(*args)
    res = {n: np.asarray(outs[i]) for i, n in enumerate(out_names)}
    return np.asarray(res["out"], np.float32)



# revision 3
# speedup vs baseline: 1.0768x; 1.0768x over previous
"""Trainium2 Bass kernel for nn_Decoder (LSTM, B=128 T=512 H=1024 O=128).

Strategy: the T=512 recurrence is inherently sequential and one step's
recurrent matmul (h @ W_hh.T: 128x1024x4096) already saturates a single
NeuronCore's PE for ~9.5us, while any cross-core exchange of h costs an
8-core AllGather floor of ~5us + HBM bounces per step. Tensor-parallel
sharding therefore cannot beat replication, so every core runs the full
recurrence (weights and state replicated); the output is taken from core 0.

Per step (on each core):
  gates = [hT;x_t;1].T @ [W_hh.T; w_ih; b]   in bf16 on the PE,
          accumulated fp32 in PSUM, N=512 tiles, K tiled 8x128 (+K=2 aug).
  Gate columns are host-permuted per 128-wide H-chunk as [i|f|o|g] so one
  strided sigmoid covers i,f,o of a chunk pair and one tanh covers g.
  c (fp32) and h (bf16) updated on DVE; tanh/sigmoid on ACT;
  h chunks transposed back to lhsT layout [H,B] via the DMA xbar (2-byte).
"""

import os
import sys

sys.path.insert(0, "/opt/trn_rl_repo")
os.environ.setdefault("JAX_PLATFORMS", "")

from contextlib import ExitStack

import numpy as np
import ml_dtypes

import concourse.bass as bass
import concourse.mybir as mybir
import concourse.tile as tile
from concourse.bass import ds
from concourse.bass_utils import run_bass_kernel_spmd

B, T, H, O = 128, 512, 1024, 128
KC = H // 128          # 8 K-tiles of the contraction over H
NCH = H // 128         # 8 H-chunks of 128 hidden units
GW = 512               # gate columns per H-chunk: [i|f|o|g] x 128
BF16 = mybir.dt.bfloat16
F32 = mybir.dt.float32

_N_CORES = int(os.environ.get("KERNEL_N_CORES", "1"))
_UNROLL = int(os.environ.get("KERNEL_UNROLL", "4"))  # steps per For_i body (even)


# ---------------------------------------------------------------- drain patch
# walrus codegen limit: InstDrain on the SP engine accepts a single sync-wait
# command, but TileContext's exit drain aggregates one wait per outstanding
# logical processor onto one drain. Split them across a chain of drains.
def _apply_drain_patch():
    import concourse.tile as _tile
    from concourse.vector_clock import ScopedClock as _ScopedClock

    if getattr(_tile.TileContext, "_drain_patch_applied", False):
        return

    def _patched(self, tick_clock, wait_clock):
        drain_inst = self.nc.sync.drain()
        wait_clock.add_sem_waits(
            drain_inst.ins, _ScopedClock({None: tick_clock.global_clock})
        )
        si = drain_inst.ins.sync_info
        waits = list(si.on_wait) if si is not None and si.on_wait else []
        if len(waits) > 1:
            si.on_wait = waits[:1]
            for w in waits[1:]:
                extra = self.nc.sync.drain()
                extra.ins.sync_info = mybir.SyncInfo(on_wait=[w], on_update=[])
        self.nc.all_engine_barrier()
        assert self.sems is not None
        popped = self.nc._tile_sem_poison_stack.pop()
        assert popped is self._sem_poison
        self.nc.clear_and_free_semaphores(list(self.sems.allocated().values()))
        self.nc.all_engine_barrier()

    _tile.TileContext._drain_and_barrier = _patched
    _tile.TileContext._drain_patch_applied = True


# ----------------------------------------------------- wait-splitting post-pass
# This walrus build accepts at most 2 sync-wait commands on ordinary engine
# instructions and only 1 on SP/TPB_CTRL-class instructions (Drain, SP DMA
# triggers). Tile attaches up to ~4. Split the excess onto InstNoOp carriers
# inserted immediately before the offending instruction on the same engine.
_SP_LIKE = ("SP",)


def _wait_limit(inst):
    # empirically: TPB_CTRL (Drain) and S3S3D3_TT (TensorTensor) templates
    # accept a single sync-wait; play safe and allow one everywhere.
    return 1


def _split_excess_waits(nc):
    n_added = 0
    for f in nc.m.functions:
        for bb in f.blocks:
            insts = bb.instructions
            out = []
            changed = False
            for inst in insts:
                si = inst.sync_info
                waits = list(si.on_wait) if si is not None and si.on_wait else []
                lim = _wait_limit(inst)
                if len(waits) > lim:
                    keep = waits[len(waits) - lim :]
                    rest = waits[: len(waits) - lim]
                    nop_lim = 1
                    while rest:
                        chunk, rest = rest[:nop_lim], rest[nop_lim:]
                        nop = mybir.InstNoOp(
                            name=f"waitnop-{n_added}", ins=[], outs=[]
                        )
                        nop.engine = inst.engine
                        nop.sync_info = mybir.SyncInfo(on_wait=chunk, on_update=[])
                        out.append(nop)
                        n_added += 1
                    si.on_wait = keep
                    changed = True
                out.append(inst)
            if changed:
                bb.instructions = out
    return n_added


# ------------------------------------------------------------- program build
def build_program(t_steps=T, unroll=_UNROLL, debug_state=False, split_waits=True):
    _apply_drain_patch()
    assert t_steps % unroll == 0 and unroll % 2 == 0
    nc = bass.Bass("TRN2", debug=False)

    wt_d = nc.dram_tensor("wt", (H, 4 * H), BF16, kind="ExternalInput").ap()
    wb_d = nc.dram_tensor("wb", (128, 4 * H), BF16, kind="ExternalInput").ap()
    bb_d = nc.dram_tensor("bb", (128, 4 * H), F32, kind="ExternalInput").ap()
    xct_d = nc.dram_tensor("xct", (B, T), F32, kind="ExternalInput").ap()
    ht0_d = nc.dram_tensor("ht0", (H, B), BF16, kind="ExternalInput").ap()
    c0_d = nc.dram_tensor("c0", (B, H), F32, kind="ExternalInput").ap()
    fcw_d = nc.dram_tensor("fcw", (128, H), BF16, kind="ExternalInput").ap()
    fca_d = nc.dram_tensor("fca", (2, 128), BF16, kind="ExternalInput").ap()
    id_d = nc.dram_tensor("ident", (128, 128), BF16, kind="ExternalInput").ap()
    out_d = nc.dram_tensor("out", (B, O), F32, kind="ExternalOutput").ap()
    if debug_state:
        ht_dbg_d = nc.dram_tensor(
            "ht_dbg", (NCH, 128, B), BF16, kind="ExternalOutput"
        ).ap()
        c_dbg_d = nc.dram_tensor("c_dbg", (B, H), F32, kind="ExternalOutput").ap()

    with tile.TileContext(nc) as tc:
        with ExitStack() as ctx:
            consts = ctx.enter_context(tc.tile_pool(name="consts", bufs=1))
            state = ctx.enter_context(tc.tile_pool(name="state", bufs=1))
            work = ctx.enter_context(tc.tile_pool(name="work", bufs=int(os.environ.get("WORK_BUFS", "3"))))
            xap = ctx.enter_context(tc.tile_pool(name="xap", bufs=int(os.environ.get("XA_BUFS", "4"))))
            psum = ctx.enter_context(tc.tile_pool(name="psum", bufs=3, space="PSUM"))
            fcp = ctx.enter_context(tc.tile_pool(name="fcp", bufs=1, space="PSUM"))
            ptp = ctx.enter_context(tc.tile_pool(name="ptp", bufs=1, space="PSUM"))


            # resident weights
            w_sb = []
            for k in range(KC):
                w_k = consts.tile([128, 4 * H], BF16, tag=f"w{k}", name=f"w{k}")
                nc.gpsimd.dma_start(out=w_k, in_=wt_d[k * 128 : (k + 1) * 128, :])
                w_sb.append(w_k)
            wb = consts.tile([128, 4 * H], BF16, tag="wb")
            nc.gpsimd.dma_start(out=wb, in_=wb_d)
            bb = consts.tile([128, 4 * H], F32, tag="bb")
            nc.gpsimd.dma_start(out=bb, in_=bb_d)
            fcw = consts.tile([128, H], BF16, tag="fcw")
            nc.gpsimd.dma_start(out=fcw, in_=fcw_d)
            ident = consts.tile([128, 128], BF16, tag="ident")
            nc.gpsimd.dma_start(out=ident, in_=id_d)
            fcb_t = consts.tile([1, 128], BF16, tag="fcb_t")
            nc.gpsimd.dma_start(out=fcb_t, in_=fca_d[0:1, :])
            ones_t = consts.tile([1, 128], BF16, tag="ones_t")
            nc.gpsimd.dma_start(out=ones_t, in_=fca_d[1:2, :])

            # state: hT ping-pong chunk tiles, fp32 cell
            ht_a = [state.tile([128, B], BF16, tag=f"hta{k}", name=f"hta{k}") for k in range(NCH)]
            ht_b = [state.tile([128, B], BF16, tag=f"htb{k}", name=f"htb{k}") for k in range(NCH)]
            c_sb = state.tile([B, H], F32, tag="c")
            for k in range(NCH):
                nc.gpsimd.dma_start(
                    out=ht_a[k], in_=ht0_d[k * 128 : (k + 1) * 128, :]
                )
            nc.gpsimd.dma_start(out=c_sb, in_=c0_d)

            def step(iv_base, local_t, cur, nxt):
                """One LSTM step. iv_base: ScalarValue or int (loop index of the
                body start); local_t: python int offset within the body."""
                xc = xap.tile([B, 1], F32, tag="xa", name="xc")
                # inside the For_i body only HWDGE DMAs are usable: the loop
                # reset block emits InstIncSwdgeSem for SWDGE queues, which
                # this walrus cannot encode ("ISA wrong length").
                if isinstance(iv_base, int):
                    off = iv_base + local_t
                    nc.sync.dma_start(out=xc, in_=xct_d[:, off : off + 1])
                else:
                    off = iv_base + local_t
                    nc.sync.dma_start(out=xc, in_=xct_d[:, ds(off, 1)])

                n_pairs = 3 if os.environ.get("TAIL_SINGLE", "0") == "1" else 4
                for p in range(n_pairs):  # pairs of H-chunks
                    ps = psum.tile([B, 2 * GW], F32, tag="gates", name=f"ps{p}")
                    cols = slice(p * 2 * GW, (p + 1) * 2 * GW)
                    # seed psum with x_t * w_ih + b on ACT/DVE (saves the
                    # K=2 aug matmul's 512 streamed columns per gate block)
                    nc.scalar.activation(
                        ps, wb[:, cols],
                        mybir.ActivationFunctionType.Copy, scale=xc,
                    )
                    nc.vector.tensor_add(ps, ps, bb[:, cols])
                    if p == 0:
                        # first block of the step: interleave the two halves
                        # so chunks 6,7 (produced at the END of the previous
                        # step) are consumed at slots 13-16 of 16, covering
                        # the prior step's eltwise+transpose tail.
                        order = [(h, k) for h in range(2) for k in range(6)] +                                 [(h, k) for k in (6, 7) for h in range(2)]
                    else:
                        order = [(h, k) for h in range(2) for k in range(KC)]
                    last = order[-1]
                    done = {0: 0, 1: 0}
                    for h_, k in order:
                        cc = 2 * p + h_
                        sl = ps[:, h_ * GW : (h_ + 1) * GW]
                        done[h_] += 1
                        nc.tensor.matmul(
                            sl,
                            lhsT=cur[k],
                            rhs=w_sb[k][:, cc * GW : (cc + 1) * GW],
                            start=False,
                            stop=(done[h_] == KC),
                            skip_group_check=True,
                        )
                    # eltwise; psum layout [i0 f0 o0 g0 i1 f1 o1 g1]
                    ps3 = ps.rearrange("p (c x) -> p c x", c=2)
                    sig = work.tile([B, 2, 384], BF16, tag="sig", name="sig")
                    nc.scalar.activation(
                        sig, ps3[:, :, 0:384], mybir.ActivationFunctionType.Sigmoid
                    )
                    tg = work.tile([B, 2, 128], BF16, tag="tg", name="tg")
                    nc.scalar.activation(
                        tg, ps3[:, :, 384:512], mybir.ActivationFunctionType.Tanh
                    )
                    sig_i = sig[:, :, 0:128]
                    sig_f = sig[:, :, 128:256]
                    sig_o = sig[:, :, 256:384]
                    c3 = c_sb[:, p * 256 : (p + 1) * 256].rearrange(
                        "p (c x) -> p c x", c=2
                    )
                    t1 = work.tile([B, 2, 128], F32, tag="t1", name="t1")
                    nc.vector.tensor_mul(t1, sig_f, c3)
                    t2 = work.tile([B, 2, 128], BF16, tag="t2", name="t2")
                    nc.vector.tensor_mul(t2, sig_i, tg)
                    nc.vector.tensor_add(c3, t1, t2)
                    tanc = work.tile([B, 2, 128], BF16, tag="tanc", name="tanc")
                    nc.scalar.activation(
                        tanc, c3, mybir.ActivationFunctionType.Tanh
                    )
                    hbf = work.tile([B, 2, 128], BF16, tag="hbf", name="hbf")
                    nc.vector.tensor_mul(hbf, sig_o, tanc)
                    for half in range(2):
                        if p >= 2:
                            # last pair is on the next step's critical path:
                            # PE transpose (~0.4us) beats the DMA xbar (~1.3us)
                            pt = ptp.tile([128, B], BF16, tag="pt", name="pt")
                            nc.tensor.transpose(pt, hbf[:, half, :], ident)
                            nc.vector.tensor_copy(nxt[2 * p + half], pt)
                        else:
                            nc.sync.dma_start_transpose(
                                nxt[2 * p + half], hbf[:, half, :]
                            )

                for cc in range(2 * n_pairs, NCH):  # tail chunks, single width
                    ps1 = psum.tile([B, GW], F32, tag="gates", name=f"ps1_{cc}")
                    cols1 = slice(cc * GW, (cc + 1) * GW)
                    nc.scalar.activation(
                        ps1, wb[:, cols1],
                        mybir.ActivationFunctionType.Copy, scale=xc,
                    )
                    nc.vector.tensor_add(ps1, ps1, bb[:, cols1])
                    for k in range(KC):
                        nc.tensor.matmul(
                            ps1, lhsT=cur[k],
                            rhs=w_sb[k][:, cc * GW : (cc + 1) * GW],
                            start=False, stop=(k == KC - 1),
                            skip_group_check=True,
                        )
                    sig1 = work.tile([B, 384], BF16, tag="sig1", name="sig1")
                    nc.scalar.activation(
                        sig1, ps1[:, 0:384], mybir.ActivationFunctionType.Sigmoid
                    )
                    tg1 = work.tile([B, 128], BF16, tag="tg1", name="tg1")
                    nc.scalar.activation(
                        tg1, ps1[:, 384:512], mybir.ActivationFunctionType.Tanh
                    )
                    c1 = c_sb[:, cc * 128 : (cc + 1) * 128]
                    t1s = work.tile([B, 128], F32, tag="t1s", name="t1s")
                    nc.vector.tensor_mul(t1s, sig1[:, 128:256], c1)
                    t2s = work.tile([B, 128], BF16, tag="t2s", name="t2s")
                    nc.vector.tensor_mul(t2s, sig1[:, 0:128], tg1)
                    nc.vector.tensor_add(c1, t1s, t2s)
                    tanc1 = work.tile([B, 128], BF16, tag="tanc1", name="tanc1")
                    nc.scalar.activation(
                        tanc1, c1, mybir.ActivationFunctionType.Tanh
                    )
                    hbf1 = work.tile([B, 128], BF16, tag="hbf1", name="hbf1")
                    nc.vector.tensor_mul(hbf1, sig1[:, 256:384], tanc1)
                    pt1 = ptp.tile([128, B], BF16, tag="pt", name="pt1")
                    nc.tensor.transpose(pt1, hbf1, ident)
                    nc.vector.tensor_copy(nxt[cc], pt1)

            if t_steps == 0:
                pass
            elif t_steps <= unroll:
                repeat_u = int(os.environ.get("KERNEL_REPEAT", "1"))

                def unrolled_body():
                    for t in range(t_steps):
                        cur, nxt = (ht_a, ht_b) if t % 2 == 0 else (ht_b, ht_a)
                        step(0, t, cur, nxt)

                if repeat_u == 1:
                    unrolled_body()
                else:
                    with tc.For_i(0, repeat_u, 1):
                        unrolled_body()
            else:
                hints = tuple(mybir.ALL_ENGINES) if os.environ.get("HINTS", "0") == "1" else ()
                repeat = int(os.environ.get("KERNEL_REPEAT", "1"))

                def inner_loop():
                    with tc.For_i(0, t_steps, unroll, hint_engines=hints) as iv:
                        for j in range(unroll):
                            cur, nxt = (ht_a, ht_b) if j % 2 == 0 else (ht_b, ht_a)
                            step(iv, j, cur, nxt)

                if repeat == 1:
                    inner_loop()
                else:  # timing amplification only: state re-evolves from t=0 xs
                    with tc.For_i(0, repeat, 1):
                        inner_loop()

            ht_fin = ht_a if t_steps % 2 == 0 else ht_b

            # final FC: out = h_T @ fc_W.T + fc_b
            fc_ps = fcp.tile([B, O], F32, tag="fc", name="fcps")
            nc.tensor.matmul(
                fc_ps, lhsT=ones_t, rhs=fcb_t, start=True, stop=False
            )
            for k in range(KC):
                nc.tensor.matmul(
                    fc_ps,
                    lhsT=ht_fin[k],
                    rhs=fcw[:, k * 128 : (k + 1) * 128],
                    start=False,
                    stop=(k == KC - 1),
                )
            out_sb = work.tile([B, O], F32, tag="out_sb")
            nc.vector.tensor_copy(out_sb, fc_ps)
            nc.gpsimd.dma_start(out=out_d, in_=out_sb)

            if debug_state:
                for k in range(NCH):
                    nc.gpsimd.dma_start(out=ht_dbg_d[k], in_=ht_fin[k])
                nc.gpsimd.dma_start(out=c_dbg_d, in_=c_sb)

    if split_waits:  # required for walrus codegen; CoreSim chokes on it
        _split_excess_waits(nc)
    return nc


# ------------------------------------------------------------------ host prep
def _prep_inputs(y_hist, W_ih, W_hh, b_ih, b_hh, fc_W, fc_b, h0, c0):
    f32 = np.float32
    bf16 = ml_dtypes.bfloat16
    # per-chunk gate permutation of the 4H rows: [i_c | f_c | o_c | g_c]
    # reference gate order in rows of W_hh is (i, f, g, o) blocks of H
    perm = np.concatenate(
        [
            g * H + c * 128 + np.arange(128)
            for c in range(NCH)
            for g in (0, 1, 3, 2)
        ]
    )
    wt = np.ascontiguousarray(W_hh[perm, :].T).astype(bf16)          # (H, 4H)
    wb = np.broadcast_to(W_ih[:, 0][perm].astype(bf16)[None, :], (128, 4 * H))
    bbv = (b_ih + b_hh)[perm].astype(f32)
    bb = np.broadcast_to(bbv[None, :], (128, 4 * H))
    xct = np.ascontiguousarray(y_hist).astype(f32)                    # (B, T)
    ht0 = np.ascontiguousarray(h0.T).astype(bf16)                     # (H, B)
    fcw = np.ascontiguousarray(fc_W.T).astype(bf16)                  # (H, O)
    # device layout for fcw tile: (128, H) with chunk k at cols [128k:128k+128)
    fcw_tile = fcw.reshape(KC, 128, O).transpose(1, 0, 2).reshape(128, H)
    fca = np.stack([fc_b, np.ones(O, f32)]).astype(bf16)              # rhs, ones
    ident = np.eye(128, dtype=f32).astype(bf16)
    return {
        "ident": np.asarray(ident),
        "wt": np.asarray(wt),
        "wb": np.ascontiguousarray(wb),
        "bb": np.ascontiguousarray(bb),
        "xct": xct,
        "ht0": np.asarray(ht0),
        "c0": c0.astype(f32),
        "fcw": np.asarray(fcw_tile),
        "fca": np.asarray(fca),
    }


_CACHE = {}


def _make_runner(nc):
    """Single-core reusable jitted executor (mirrors bass2jax.run_bass_via_pjrt
    but caches the jitted body so repeated kernel() calls skip retracing)."""
    import jax
    from concourse import bass2jax

    bass2jax.install_neuronx_cc_hook()
    partition_name = nc.partition_id_tensor.name if nc.partition_id_tensor else None
    in_names, out_names, out_avals, zero_outs = [], [], [], []
    for alloc in nc.m.functions[0].allocations:
        if not isinstance(alloc, mybir.MemoryLocationSet):
            continue
        name = alloc.memorylocations[0].name
        if alloc.kind == "ExternalInput":
            if name != partition_name:
                in_names.append(name)
        elif alloc.kind == "ExternalOutput":
            shape = tuple(alloc.tensor_shape)
            dtype = mybir.dt.np(alloc.dtype)
            out_names.append(name)
            out_avals.append(jax.core.ShapedArray(shape, dtype))
            zero_outs.append(np.zeros(shape, dtype))
    all_in = list(in_names) + list(out_names)
    if partition_name is not None:
        all_in.append(partition_name)

    def _body(*args):
        operands = list(args)
        if partition_name is not None:
            operands.append(bass2jax.partition_id_tensor())
        return tuple(
            bass2jax._bass_exec_p.bind(
                *operands,
                out_avals=tuple(out_avals),
                in_names=tuple(all_in),
                out_names=tuple(out_names),
                lowering_input_output_aliases=(),
                sim_require_finite=True,
                sim_require_nnan=True,
                nc=nc,
            )
        )

    f = jax.jit(_body, keep_unused=True)
    return f, in_names, out_names, zero_outs


def kernel(y_hist, W_ih, W_hh, b_ih, b_hh, fc_W, fc_b, h0, c0, **kw):
    dev_in = _prep_inputs(
        np.asarray(y_hist, np.float32),
        np.asarray(W_ih, np.float32),
        np.asarray(W_hh, np.float32),
        np.asarray(b_ih, np.float32),
        np.asarray(b_hh, np.float32),
        np.asarray(fc_W, np.float32),
        np.asarray(fc_b, np.float32),
        np.asarray(h0, np.float32),
        np.asarray(c0, np.float32),
    )
    if _N_CORES != 1:
        if "nc" not in _CACHE:
            _CACHE["nc"] = build_program()
        res = run_bass_kernel_spmd(
            _CACHE["nc"],
            [dict(dev_in) for _ in range(_N_CORES)],
            core_ids=list(range(_N_CORES)),
        )
        return np.asarray(res.results[0]["out"], np.float32)
    if "runner" not in _CACHE:
        nc = build_program()
        _CACHE["runner"] = _make_runner(nc)
    f, in_names, out_names, zero_outs = _CACHE["runner"]
    args = [np.asarray(dev_in[n]) for n in in_names] + zero_outs
    outs = f(*args)
    res = {n: np.asarray(outs[i]) for i, n in enumerate(out_names)}
    return np.asarray(res["out"], np.float32)



# revision 5
# speedup vs baseline: 1.2464x; 1.1575x over previous
"""Trainium2 Bass kernel for nn_Decoder (LSTM, B=128 T=512 H=1024 O=128).

Strategy: the T=512 recurrence is inherently sequential and one step's
recurrent matmul (h @ W_hh.T: 128x1024x4096) already saturates a single
NeuronCore's PE for ~9.5us, while any cross-core exchange of h costs an
8-core AllGather floor of ~5us + HBM bounces per step. Tensor-parallel
sharding therefore cannot beat replication, so every core runs the full
recurrence (weights and state replicated); the output is taken from core 0.

Per step (on each core):
  gates = [hT;x_t;1].T @ [W_hh.T; w_ih; b]   in bf16 on the PE,
          accumulated fp32 in PSUM, N=512 tiles, K tiled 8x128 (+K=2 aug).
  Gate columns are host-permuted per 128-wide H-chunk as [i|f|o|g] so one
  strided sigmoid covers i,f,o of a chunk pair and one tanh covers g.
  c (fp32) and h (bf16) updated on DVE; tanh/sigmoid on ACT;
  h chunks transposed back to lhsT layout [H,B] via the DMA xbar (2-byte).
"""

import os
import sys

sys.path.insert(0, "/opt/trn_rl_repo")
os.environ.setdefault("JAX_PLATFORMS", "")

from contextlib import ExitStack

import numpy as np
import ml_dtypes

import concourse.bass as bass
import concourse.mybir as mybir
import concourse.tile as tile
from concourse.bass import ds
from concourse.bass_utils import run_bass_kernel_spmd

B, T, H, O = 128, 512, 1024, 128
KC = H // 128          # 8 K-tiles of the contraction over H
NCH = H // 128         # 8 H-chunks of 128 hidden units
GW = 512               # gate columns per H-chunk: [i|f|o|g] x 128
BF16 = mybir.dt.bfloat16
F32 = mybir.dt.float32

_N_CORES = int(os.environ.get("KERNEL_N_CORES", "1"))
_UNROLL = int(os.environ.get("KERNEL_UNROLL", "16"))  # steps per For_i body (even)


# ---------------------------------------------------------------- drain patch
# walrus codegen limit: InstDrain on the SP engine accepts a single sync-wait
# command, but TileContext's exit drain aggregates one wait per outstanding
# logical processor onto one drain. Split them across a chain of drains.
def _apply_drain_patch():
    import concourse.tile as _tile
    from concourse.vector_clock import ScopedClock as _ScopedClock

    if getattr(_tile.TileContext, "_drain_patch_applied", False):
        return

    def _patched(self, tick_clock, wait_clock):
        drain_inst = self.nc.sync.drain()
        wait_clock.add_sem_waits(
            drain_inst.ins, _ScopedClock({None: tick_clock.global_clock})
        )
        si = drain_inst.ins.sync_info
        waits = list(si.on_wait) if si is not None and si.on_wait else []
        if len(waits) > 1:
            si.on_wait = waits[:1]
            for w in waits[1:]:
                extra = self.nc.sync.drain()
                extra.ins.sync_info = mybir.SyncInfo(on_wait=[w], on_update=[])
        self.nc.all_engine_barrier()
        assert self.sems is not None
        popped = self.nc._tile_sem_poison_stack.pop()
        assert popped is self._sem_poison
        self.nc.clear_and_free_semaphores(list(self.sems.allocated().values()))
        self.nc.all_engine_barrier()

    _tile.TileContext._drain_and_barrier = _patched
    _tile.TileContext._drain_patch_applied = True


# ----------------------------------------------------- wait-splitting post-pass
# This walrus build accepts at most 2 sync-wait commands on ordinary engine
# instructions and only 1 on SP/TPB_CTRL-class instructions (Drain, SP DMA
# triggers). Tile attaches up to ~4. Split the excess onto InstNoOp carriers
# inserted immediately before the offending instruction on the same engine.
_SP_LIKE = ("SP",)


def _wait_limit(inst):
    # empirically: TPB_CTRL (Drain) and S3S3D3_TT (TensorTensor) templates
    # accept a single sync-wait; play safe and allow one everywhere.
    return 1


def _split_excess_waits(nc):
    n_added = 0
    for f in nc.m.functions:
        for bb in f.blocks:
            insts = bb.instructions
            out = []
            changed = False
            for inst in insts:
                si = inst.sync_info
                waits = list(si.on_wait) if si is not None and si.on_wait else []
                lim = _wait_limit(inst)
                if len(waits) > lim:
                    keep = waits[len(waits) - lim :]
                    rest = waits[: len(waits) - lim]
                    nop_lim = 1
                    while rest:
                        chunk, rest = rest[:nop_lim], rest[nop_lim:]
                        nop = mybir.InstNoOp(
                            name=f"waitnop-{n_added}", ins=[], outs=[]
                        )
                        nop.engine = inst.engine
                        nop.sync_info = mybir.SyncInfo(on_wait=chunk, on_update=[])
                        out.append(nop)
                        n_added += 1
                    si.on_wait = keep
                    changed = True
                out.append(inst)
            if changed:
                bb.instructions = out
    return n_added


# ------------------------------------------------------------- program build
def build_program(t_steps=T, unroll=_UNROLL, debug_state=False, split_waits=True):
    _apply_drain_patch()
    assert t_steps % unroll == 0 and unroll % 2 == 0
    nc = bass.Bass("TRN2", debug=False)

    wt_d = nc.dram_tensor("wt", (H, 4 * H), BF16, kind="ExternalInput").ap()
    wb_d = nc.dram_tensor("wb", (128, 4 * H), BF16, kind="ExternalInput").ap()
    bb_d = nc.dram_tensor("bb", (128, 4 * H), F32, kind="ExternalInput").ap()
    xct_d = nc.dram_tensor("xct", (B, T), F32, kind="ExternalInput").ap()
    ht0_d = nc.dram_tensor("ht0", (H, B), BF16, kind="ExternalInput").ap()
    c0_d = nc.dram_tensor("c0", (B, H), F32, kind="ExternalInput").ap()
    fcw_d = nc.dram_tensor("fcw", (128, H), BF16, kind="ExternalInput").ap()
    fca_d = nc.dram_tensor("fca", (2, 128), BF16, kind="ExternalInput").ap()
    id_d = nc.dram_tensor("ident", (128, 128), BF16, kind="ExternalInput").ap()
    out_d = nc.dram_tensor("out", (B, O), F32, kind="ExternalOutput").ap()
    if debug_state:
        ht_dbg_d = nc.dram_tensor(
            "ht_dbg", (NCH, 128, B), BF16, kind="ExternalOutput"
        ).ap()
        c_dbg_d = nc.dram_tensor("c_dbg", (B, H), F32, kind="ExternalOutput").ap()

    with tile.TileContext(nc) as tc:
        with ExitStack() as ctx:
            consts = ctx.enter_context(tc.tile_pool(name="consts", bufs=1))
            state = ctx.enter_context(tc.tile_pool(name="state", bufs=1))
            work = ctx.enter_context(tc.tile_pool(name="work", bufs=int(os.environ.get("WORK_BUFS", "3"))))
            xap = ctx.enter_context(tc.tile_pool(name="xap", bufs=int(os.environ.get("XA_BUFS", "4"))))
            psum = ctx.enter_context(tc.tile_pool(name="psum", bufs=3, space="PSUM"))
            fcp = ctx.enter_context(tc.tile_pool(name="fcp", bufs=1, space="PSUM"))
            ptp = ctx.enter_context(tc.tile_pool(name="ptp", bufs=1, space="PSUM"))


            # resident weights
            w_sb = []
            for k in range(KC):
                w_k = consts.tile([128, 4 * H], BF16, tag=f"w{k}", name=f"w{k}")
                nc.gpsimd.dma_start(out=w_k, in_=wt_d[k * 128 : (k + 1) * 128, :])
                w_sb.append(w_k)
            wb = consts.tile([128, 4 * H], BF16, tag="wb")
            nc.gpsimd.dma_start(out=wb, in_=wb_d)
            bb = consts.tile([128, 4 * H], F32, tag="bb")
            nc.gpsimd.dma_start(out=bb, in_=bb_d)
            fcw = consts.tile([128, H], BF16, tag="fcw")
            nc.gpsimd.dma_start(out=fcw, in_=fcw_d)
            ident = consts.tile([128, 128], BF16, tag="ident")
            nc.gpsimd.dma_start(out=ident, in_=id_d)
            fcb_t = consts.tile([1, 128], BF16, tag="fcb_t")
            nc.gpsimd.dma_start(out=fcb_t, in_=fca_d[0:1, :])
            ones_t = consts.tile([1, 128], BF16, tag="ones_t")
            nc.gpsimd.dma_start(out=ones_t, in_=fca_d[1:2, :])

            # state: hT ping-pong chunk tiles, fp32 cell
            ht_a = [state.tile([128, B], BF16, tag=f"hta{k}", name=f"hta{k}") for k in range(NCH)]
            ht_b = [state.tile([128, B], BF16, tag=f"htb{k}", name=f"htb{k}") for k in range(NCH)]
            c_sb = state.tile([B, H], F32, tag="c")
            for k in range(NCH):
                nc.gpsimd.dma_start(
                    out=ht_a[k], in_=ht0_d[k * 128 : (k + 1) * 128, :]
                )
            nc.gpsimd.dma_start(out=c_sb, in_=c0_d)

            def step(iv_base, local_t, cur, nxt):
                """One LSTM step. iv_base: ScalarValue or int (loop index of the
                body start); local_t: python int offset within the body."""
                xc = xap.tile([B, 1], F32, tag="xa", name="xc")
                # inside the For_i body only HWDGE DMAs are usable: the loop
                # reset block emits InstIncSwdgeSem for SWDGE queues, which
                # this walrus cannot encode ("ISA wrong length").
                if isinstance(iv_base, int):
                    off = iv_base + local_t
                    nc.sync.dma_start(out=xc, in_=xct_d[:, off : off + 1])
                else:
                    off = iv_base + local_t
                    nc.sync.dma_start(out=xc, in_=xct_d[:, ds(off, 1)])

                n_pairs = 3 if os.environ.get("TAIL_SINGLE", "0") == "1" else 4
                for p in range(n_pairs):  # pairs of H-chunks
                    ps = psum.tile([B, 2 * GW], F32, tag="gates", name=f"ps{p}")
                    cols = slice(p * 2 * GW, (p + 1) * 2 * GW)
                    # seed psum with x_t * w_ih + b on ACT/DVE (saves the
                    # K=2 aug matmul's 512 streamed columns per gate block)
                    nc.scalar.activation(
                        ps, wb[:, cols],
                        mybir.ActivationFunctionType.Copy, scale=xc,
                    )
                    nc.vector.tensor_add(ps, ps, bb[:, cols])
                    if p == 0:
                        # first block of the step: interleave the two halves
                        # so chunks 6,7 (produced at the END of the previous
                        # step) are consumed at slots 13-16 of 16, covering
                        # the prior step's eltwise+transpose tail.
                        order = [(h, k) for h in range(2) for k in range(6)] +                                 [(h, k) for k in (6, 7) for h in range(2)]
                    else:
                        order = [(h, k) for h in range(2) for k in range(KC)]
                    last = order[-1]
                    done = {0: 0, 1: 0}
                    for h_, k in order:
                        cc = 2 * p + h_
                        sl = ps[:, h_ * GW : (h_ + 1) * GW]
                        done[h_] += 1
                        nc.tensor.matmul(
                            sl,
                            lhsT=cur[k],
                            rhs=w_sb[k][:, cc * GW : (cc + 1) * GW],
                            start=False,
                            stop=(done[h_] == KC),
                            skip_group_check=True,
                        )
                    # eltwise; psum layout [i0 f0 o0 g0 i1 f1 o1 g1]
                    ps3 = ps.rearrange("p (c x) -> p c x", c=2)
                    sig = work.tile([B, 2, 384], BF16, tag="sig", name="sig")
                    nc.scalar.activation(
                        sig, ps3[:, :, 0:384], mybir.ActivationFunctionType.Sigmoid
                    )
                    tg = work.tile([B, 2, 128], BF16, tag="tg", name="tg")
                    nc.scalar.activation(
                        tg, ps3[:, :, 384:512], mybir.ActivationFunctionType.Tanh
                    )
                    sig_i = sig[:, :, 0:128]
                    sig_f = sig[:, :, 128:256]
                    sig_o = sig[:, :, 256:384]
                    c3 = c_sb[:, p * 256 : (p + 1) * 256].rearrange(
                        "p (c x) -> p c x", c=2
                    )
                    t1 = work.tile([B, 2, 128], F32, tag="t1", name="t1")
                    nc.vector.tensor_mul(t1, sig_f, c3)
                    t2 = work.tile([B, 2, 128], BF16, tag="t2", name="t2")
                    nc.vector.tensor_mul(t2, sig_i, tg)
                    nc.vector.tensor_add(c3, t1, t2)
                    tanc = work.tile([B, 2, 128], BF16, tag="tanc", name="tanc")
                    nc.scalar.activation(
                        tanc, c3, mybir.ActivationFunctionType.Tanh
                    )
                    hbf = work.tile([B, 2, 128], BF16, tag="hbf", name="hbf")
                    nc.vector.tensor_mul(hbf, sig_o, tanc)
                    for half in range(2):
                        if p >= int(os.environ.get("PE_T_MIN", "0")):
                            # last pair is on the next step's critical path:
                            # PE transpose (~0.4us) beats the DMA xbar (~1.3us)
                            pt = ptp.tile([128, B], BF16, tag="pt", name="pt")
                            nc.tensor.transpose(pt, hbf[:, half, :], ident)
                            nc.vector.tensor_copy(nxt[2 * p + half], pt)
                        else:
                            nc.sync.dma_start_transpose(
                                nxt[2 * p + half], hbf[:, half, :]
                            )

                for cc in range(2 * n_pairs, NCH):  # tail chunks, single width
                    ps1 = psum.tile([B, GW], F32, tag="gates", name=f"ps1_{cc}")
                    cols1 = slice(cc * GW, (cc + 1) * GW)
                    nc.scalar.activation(
                        ps1, wb[:, cols1],
                        mybir.ActivationFunctionType.Copy, scale=xc,
                    )
                    nc.vector.tensor_add(ps1, ps1, bb[:, cols1])
                    for k in range(KC):
                        nc.tensor.matmul(
                            ps1, lhsT=cur[k],
                            rhs=w_sb[k][:, cc * GW : (cc + 1) * GW],
                            start=False, stop=(k == KC - 1),
                            skip_group_check=True,
                        )
                    sig1 = work.tile([B, 384], BF16, tag="sig1", name="sig1")
                    nc.scalar.activation(
                        sig1, ps1[:, 0:384], mybir.ActivationFunctionType.Sigmoid
                    )
                    tg1 = work.tile([B, 128], BF16, tag="tg1", name="tg1")
                    nc.scalar.activation(
                        tg1, ps1[:, 384:512], mybir.ActivationFunctionType.Tanh
                    )
                    c1 = c_sb[:, cc * 128 : (cc + 1) * 128]
                    t1s = work.tile([B, 128], F32, tag="t1s", name="t1s")
                    nc.vector.tensor_mul(t1s, sig1[:, 128:256], c1)
                    t2s = work.tile([B, 128], BF16, tag="t2s", name="t2s")
                    nc.vector.tensor_mul(t2s, sig1[:, 0:128], tg1)
                    nc.vector.tensor_add(c1, t1s, t2s)
                    tanc1 = work.tile([B, 128], BF16, tag="tanc1", name="tanc1")
                    nc.scalar.activation(
                        tanc1, c1, mybir.ActivationFunctionType.Tanh
                    )
                    hbf1 = work.tile([B, 128], BF16, tag="hbf1", name="hbf1")
                    nc.vector.tensor_mul(hbf1, sig1[:, 256:384], tanc1)
                    pt1 = ptp.tile([128, B], BF16, tag="pt", name="pt1")
                    nc.tensor.transpose(pt1, hbf1, ident)
                    nc.vector.tensor_copy(nxt[cc], pt1)

            if t_steps == 0:
                pass
            elif t_steps <= unroll:
                repeat_u = int(os.environ.get("KERNEL_REPEAT", "1"))

                def unrolled_body():
                    for t in range(t_steps):
                        cur, nxt = (ht_a, ht_b) if t % 2 == 0 else (ht_b, ht_a)
                        step(0, t, cur, nxt)

                if repeat_u == 1:
                    unrolled_body()
                else:
                    with tc.For_i(0, repeat_u, 1):
                        unrolled_body()
            else:
                hints = tuple(mybir.ALL_ENGINES) if os.environ.get("HINTS", "0") == "1" else ()
                repeat = int(os.environ.get("KERNEL_REPEAT", "1"))

                def inner_loop():
                    with tc.For_i(0, t_steps, unroll, hint_engines=hints) as iv:
                        for j in range(unroll):
                            cur, nxt = (ht_a, ht_b) if j % 2 == 0 else (ht_b, ht_a)
                            step(iv, j, cur, nxt)

                if repeat == 1:
                    inner_loop()
                else:  # timing amplification only: state re-evolves from t=0 xs
                    with tc.For_i(0, repeat, 1):
                        inner_loop()

            ht_fin = ht_a if t_steps % 2 == 0 else ht_b

            # final FC: out = h_T @ fc_W.T + fc_b
            fc_ps = fcp.tile([B, O], F32, tag="fc", name="fcps")
            nc.tensor.matmul(
                fc_ps, lhsT=ones_t, rhs=fcb_t, start=True, stop=False
            )
            for k in range(KC):
                nc.tensor.matmul(
                    fc_ps,
                    lhsT=ht_fin[k],
                    rhs=fcw[:, k * 128 : (k + 1) * 128],
                    start=False,
                    stop=(k == KC - 1),
                )
            out_sb = work.tile([B, O], F32, tag="out_sb")
            nc.vector.tensor_copy(out_sb, fc_ps)
            nc.gpsimd.dma_start(out=out_d, in_=out_sb)

            if debug_state:
                for k in range(NCH):
                    nc.gpsimd.dma_start(out=ht_dbg_d[k], in_=ht_fin[k])
                nc.gpsimd.dma_start(out=c_dbg_d, in_=c_sb)

    if split_waits:  # required for walrus codegen; CoreSim chokes on it
        _split_excess_waits(nc)
    return nc


# ------------------------------------------------------------------ host prep
def _prep_inputs(y_hist, W_ih, W_hh, b_ih, b_hh, fc_W, fc_b, h0, c0):
    f32 = np.float32
    bf16 = ml_dtypes.bfloat16
    # per-chunk gate permutation of the 4H rows: [i_c | f_c | o_c | g_c]
    # reference gate order in rows of W_hh is (i, f, g, o) blocks of H
    perm = np.concatenate(
        [
            g * H + c * 128 + np.arange(128)
            for c in range(NCH)
            for g in (0, 1, 3, 2)
        ]
    )
    wt = np.ascontiguousarray(W_hh[perm, :].T).astype(bf16)          # (H, 4H)
    wb = np.broadcast_to(W_ih[:, 0][perm].astype(bf16)[None, :], (128, 4 * H))
    bbv = (b_ih + b_hh)[perm].astype(f32)
    bb = np.broadcast_to(bbv[None, :], (128, 4 * H))
    xct = np.ascontiguousarray(y_hist).astype(f32)                    # (B, T)
    ht0 = np.ascontiguousarray(h0.T).astype(bf16)                     # (H, B)
    fcw = np.ascontiguousarray(fc_W.T).astype(bf16)                  # (H, O)
    # device layout for fcw tile: (128, H) with chunk k at cols [128k:128k+128)
    fcw_tile = fcw.reshape(KC, 128, O).transpose(1, 0, 2).reshape(128, H)
    fca = np.stack([fc_b, np.ones(O, f32)]).astype(bf16)              # rhs, ones
    ident = np.eye(128, dtype=f32).astype(bf16)
    return {
        "ident": np.asarray(ident),
        "wt": np.asarray(wt),
        "wb": np.ascontiguousarray(wb),
        "bb": np.ascontiguousarray(bb),
        "xct": xct,
        "ht0": np.asarray(ht0),
        "c0": c0.astype(f32),
        "fcw": np.asarray(fcw_tile),
        "fca": np.asarray(fca),
    }


_CACHE = {}


def _make_runner(nc):
    """Single-core reusable jitted executor (mirrors bass2jax.run_bass_via_pjrt
    but caches the jitted body so repeated kernel() calls skip retracing)."""
    import jax
    from concourse import bass2jax

    bass2jax.install_neuronx_cc_hook()
    partition_name = nc.partition_id_tensor.name if nc.partition_id_tensor else None
    in_names, out_names, out_avals, zero_outs = [], [], [], []
    for alloc in nc.m.functions[0].allocations:
        if not isinstance(alloc, mybir.MemoryLocationSet):
            continue
        name = alloc.memorylocations[0].name
        if alloc.kind == "ExternalInput":
            if name != partition_name:
                in_names.append(name)
        elif alloc.kind == "ExternalOutput":
            shape = tuple(alloc.tensor_shape)
            dtype = mybir.dt.np(alloc.dtype)
            out_names.append(name)
            out_avals.append(jax.core.ShapedArray(shape, dtype))
            zero_outs.append(np.zeros(shape, dtype))
    all_in = list(in_names) + list(out_names)
    if partition_name is not None:
        all_in.append(partition_name)

    def _body(*args):
        operands = list(args)
        if partition_name is not None:
            operands.append(bass2jax.partition_id_tensor())
        return tuple(
            bass2jax._bass_exec_p.bind(
                *operands,
                out_avals=tuple(out_avals),
                in_names=tuple(all_in),
                out_names=tuple(out_names),
                lowering_input_output_aliases=(),
                sim_require_finite=True,
                sim_require_nnan=True,
                nc=nc,
            )
        )

    f = jax.jit(_body, keep_unused=True)
    return f, in_names, out_names, zero_outs


def kernel(y_hist, W_ih, W_hh, b_ih, b_hh, fc_W, fc_b, h0, c0, **kw):
    dev_in = _prep_inputs(
        np.asarray(y_hist, np.float32),
        np.asarray(W_ih, np.float32),
        np.asarray(W_hh, np.float32),
        np.asarray(b_ih, np.float32),
        np.asarray(b_hh, np.float32),
        np.asarray(fc_W, np.float32),
        np.asarray(fc_b, np.float32),
        np.asarray(h0, np.float32),
        np.asarray(c0, np.float32),
    )
    if _N_CORES != 1:
        if "nc" not in _CACHE:
            _CACHE["nc"] = build_program()
        res = run_bass_kernel_spmd(
            _CACHE["nc"],
            [dict(dev_in) for _ in range(_N_CORES)],
            core_ids=list(range(_N_CORES)),
        )
        return np.asarray(res.results[0]["out"], np.float32)
    if "runner" not in _CACHE:
        nc = build_program()
        _CACHE["runner"] = _make_runner(nc)
    f, in_names, out_names, zero_outs = _CACHE["runner"]
    args = [np.asarray(dev_in[n]) for n in in_names] + zero_outs
    outs = f(*args)
    res = {n: np.asarray(outs[i]) for i, n in enumerate(out_names)}
    return np.asarray(res["out"], np.float32)



# revision 6
# speedup vs baseline: 1.3184x; 1.0578x over previous
"""Trainium2 Bass kernel for nn_Decoder (LSTM, B=128 T=512 H=1024 O=128).

Strategy: the T=512 recurrence is inherently sequential and one step's
recurrent matmul (h @ W_hh.T: 128x1024x4096) already saturates a single
NeuronCore's PE for ~9.5us, while any cross-core exchange of h costs an
8-core AllGather floor of ~5us + HBM bounces per step. Tensor-parallel
sharding therefore cannot beat replication, so every core runs the full
recurrence (weights and state replicated); the output is taken from core 0.

Per step (on each core):
  gates = [hT;x_t;1].T @ [W_hh.T; w_ih; b]   in bf16 on the PE,
          accumulated fp32 in PSUM, N=512 tiles, K tiled 8x128 (+K=2 aug).
  Gate columns are host-permuted per 128-wide H-chunk as [i|f|o|g] so one
  strided sigmoid covers i,f,o of a chunk pair and one tanh covers g.
  c (fp32) and h (bf16) updated on DVE; tanh/sigmoid on ACT;
  h chunks transposed back to lhsT layout [H,B] via the DMA xbar (2-byte).
"""

import os
import sys

sys.path.insert(0, "/opt/trn_rl_repo")
os.environ.setdefault("JAX_PLATFORMS", "")

from contextlib import ExitStack

import numpy as np
import ml_dtypes

import concourse.bass as bass
import concourse.mybir as mybir
import concourse.tile as tile
from concourse.bass import ds
from concourse.bass_utils import run_bass_kernel_spmd

B, T, H, O = 128, 512, 1024, 128
KC = H // 128          # 8 K-tiles of the contraction over H
NCH = H // 128         # 8 H-chunks of 128 hidden units
GW = 512               # gate columns per H-chunk: [i|f|o|g] x 128
BF16 = mybir.dt.bfloat16
F32 = mybir.dt.float32

_N_CORES = int(os.environ.get("KERNEL_N_CORES", "1"))
_UNROLL = int(os.environ.get("KERNEL_UNROLL", "16"))  # steps per For_i body (even)


# ---------------------------------------------------------------- drain patch
# walrus codegen limit: InstDrain on the SP engine accepts a single sync-wait
# command, but TileContext's exit drain aggregates one wait per outstanding
# logical processor onto one drain. Split them across a chain of drains.
def _apply_drain_patch():
    import concourse.tile as _tile
    from concourse.vector_clock import ScopedClock as _ScopedClock

    if getattr(_tile.TileContext, "_drain_patch_applied", False):
        return

    def _patched(self, tick_clock, wait_clock):
        drain_inst = self.nc.sync.drain()
        wait_clock.add_sem_waits(
            drain_inst.ins, _ScopedClock({None: tick_clock.global_clock})
        )
        si = drain_inst.ins.sync_info
        waits = list(si.on_wait) if si is not None and si.on_wait else []
        if len(waits) > 1:
            si.on_wait = waits[:1]
            for w in waits[1:]:
                extra = self.nc.sync.drain()
                extra.ins.sync_info = mybir.SyncInfo(on_wait=[w], on_update=[])
        self.nc.all_engine_barrier()
        assert self.sems is not None
        popped = self.nc._tile_sem_poison_stack.pop()
        assert popped is self._sem_poison
        self.nc.clear_and_free_semaphores(list(self.sems.allocated().values()))
        self.nc.all_engine_barrier()

    _tile.TileContext._drain_and_barrier = _patched
    _tile.TileContext._drain_patch_applied = True


# ----------------------------------------------------- wait-splitting post-pass
# This walrus build accepts at most 2 sync-wait commands on ordinary engine
# instructions and only 1 on SP/TPB_CTRL-class instructions (Drain, SP DMA
# triggers). Tile attaches up to ~4. Split the excess onto InstNoOp carriers
# inserted immediately before the offending instruction on the same engine.
_SP_LIKE = ("SP",)


def _wait_limit(inst):
    # empirically: TPB_CTRL (Drain) and S3S3D3_TT (TensorTensor) templates
    # accept a single sync-wait; play safe and allow one everywhere.
    return 1


def _split_excess_waits(nc):
    n_added = 0
    for f in nc.m.functions:
        for bb in f.blocks:
            insts = bb.instructions
            out = []
            changed = False
            for inst in insts:
                si = inst.sync_info
                waits = list(si.on_wait) if si is not None and si.on_wait else []
                lim = _wait_limit(inst)
                if len(waits) > lim:
                    keep = waits[len(waits) - lim :]
                    rest = waits[: len(waits) - lim]
                    nop_lim = 1
                    while rest:
                        chunk, rest = rest[:nop_lim], rest[nop_lim:]
                        nop = mybir.InstNoOp(
                            name=f"waitnop-{n_added}", ins=[], outs=[]
                        )
                        nop.engine = inst.engine
                        nop.sync_info = mybir.SyncInfo(on_wait=chunk, on_update=[])
                        out.append(nop)
                        n_added += 1
                    si.on_wait = keep
                    changed = True
                out.append(inst)
            if changed:
                bb.instructions = out
    return n_added


# ------------------------------------------------------------- program build
def build_program(t_steps=T, unroll=_UNROLL, debug_state=False, split_waits=True):
    _apply_drain_patch()
    assert t_steps % unroll == 0 and unroll % 2 == 0
    nc = bass.Bass("TRN2", debug=False)

    wt_d = nc.dram_tensor("wt", (H, 4 * H), BF16, kind="ExternalInput").ap()
    wb_d = nc.dram_tensor("wb", (128, 4 * H), BF16, kind="ExternalInput").ap()
    bb_d = nc.dram_tensor("bb", (128, 4 * H), F32, kind="ExternalInput").ap()
    xct_d = nc.dram_tensor("xct", (B, T), F32, kind="ExternalInput").ap()
    ht0_d = nc.dram_tensor("ht0", (H, B), BF16, kind="ExternalInput").ap()
    c0_d = nc.dram_tensor("c0", (B, H), F32, kind="ExternalInput").ap()
    fcw_d = nc.dram_tensor("fcw", (128, H), BF16, kind="ExternalInput").ap()
    fca_d = nc.dram_tensor("fca", (2, 128), BF16, kind="ExternalInput").ap()
    id_d = nc.dram_tensor("ident", (128, 128), BF16, kind="ExternalInput").ap()
    out_d = nc.dram_tensor("out", (B, O), F32, kind="ExternalOutput").ap()
    if debug_state:
        ht_dbg_d = nc.dram_tensor(
            "ht_dbg", (NCH, 128, B), BF16, kind="ExternalOutput"
        ).ap()
        c_dbg_d = nc.dram_tensor("c_dbg", (B, H), F32, kind="ExternalOutput").ap()

    with tile.TileContext(nc) as tc:
        with ExitStack() as ctx:
            consts = ctx.enter_context(tc.tile_pool(name="consts", bufs=1))
            state = ctx.enter_context(tc.tile_pool(name="state", bufs=1))
            work = ctx.enter_context(tc.tile_pool(name="work", bufs=int(os.environ.get("WORK_BUFS", "3"))))
            xap = ctx.enter_context(tc.tile_pool(name="xap", bufs=int(os.environ.get("XA_BUFS", "4"))))
            psum = ctx.enter_context(tc.tile_pool(name="psum", bufs=3, space="PSUM"))
            ptp = ctx.enter_context(tc.tile_pool(name="ptp", bufs=2, space="PSUM"))


            # resident weights
            w_sb = []
            for k in range(KC):
                w_k = consts.tile([128, 4 * H], BF16, tag=f"w{k}", name=f"w{k}")
                nc.gpsimd.dma_start(out=w_k, in_=wt_d[k * 128 : (k + 1) * 128, :])
                w_sb.append(w_k)
            wb = consts.tile([128, 4 * H], BF16, tag="wb")
            nc.gpsimd.dma_start(out=wb, in_=wb_d)
            bb = consts.tile([128, 4 * H], F32, tag="bb")
            nc.gpsimd.dma_start(out=bb, in_=bb_d)
            fcw = consts.tile([128, H], BF16, tag="fcw")
            nc.gpsimd.dma_start(out=fcw, in_=fcw_d)
            ident = consts.tile([128, 128], BF16, tag="ident")
            nc.gpsimd.dma_start(out=ident, in_=id_d)
            fcb_t = consts.tile([1, 128], BF16, tag="fcb_t")
            nc.gpsimd.dma_start(out=fcb_t, in_=fca_d[0:1, :])
            ones_t = consts.tile([1, 128], BF16, tag="ones_t")
            nc.gpsimd.dma_start(out=ones_t, in_=fca_d[1:2, :])

            # state: hT ping-pong chunk tiles, fp32 cell
            ht_a = [state.tile([128, B], BF16, tag=f"hta{k}", name=f"hta{k}") for k in range(NCH)]
            ht_b = [state.tile([128, B], BF16, tag=f"htb{k}", name=f"htb{k}") for k in range(NCH)]
            c_sb = state.tile([B, H], F32, tag="c")
            for k in range(NCH):
                nc.gpsimd.dma_start(
                    out=ht_a[k], in_=ht0_d[k * 128 : (k + 1) * 128, :]
                )
            nc.gpsimd.dma_start(out=c_sb, in_=c0_d)

            def step(iv_base, local_t, cur, nxt):
                """One LSTM step. iv_base: ScalarValue or int (loop index of the
                body start); local_t: python int offset within the body."""
                xc = xap.tile([B, 1], F32, tag="xa", name="xc")
                # inside the For_i body only HWDGE DMAs are usable: the loop
                # reset block emits InstIncSwdgeSem for SWDGE queues, which
                # this walrus cannot encode ("ISA wrong length").
                if isinstance(iv_base, int):
                    off = iv_base + local_t
                    nc.sync.dma_start(out=xc, in_=xct_d[:, off : off + 1])
                else:
                    off = iv_base + local_t
                    nc.sync.dma_start(out=xc, in_=xct_d[:, ds(off, 1)])

                n_pairs = 3 if os.environ.get("TAIL_SINGLE", "0") == "1" else 4
                for p in range(n_pairs):  # pairs of H-chunks
                    ps = psum.tile([B, 2 * GW], F32, tag="gates", name=f"ps{p}")
                    cols = slice(p * 2 * GW, (p + 1) * 2 * GW)
                    # seed psum with x_t * w_ih + b on ACT/DVE (saves the
                    # K=2 aug matmul's 512 streamed columns per gate block)
                    nc.scalar.activation(
                        ps, wb[:, cols],
                        mybir.ActivationFunctionType.Copy, scale=xc,
                    )
                    nc.vector.tensor_add(ps, ps, bb[:, cols])
                    if p == 0:
                        # first block of the step: interleave the two halves
                        # so chunks 6,7 (produced at the END of the previous
                        # step) are consumed at slots 13-16 of 16, covering
                        # the prior step's eltwise+transpose tail.
                        order = [(h, k) for h in range(2) for k in range(6)] +                                 [(h, k) for k in (6, 7) for h in range(2)]
                    else:
                        order = [(h, k) for h in range(2) for k in range(KC)]
                    last = order[-1]
                    done = {0: 0, 1: 0}
                    for h_, k in order:
                        cc = 2 * p + h_
                        sl = ps[:, h_ * GW : (h_ + 1) * GW]
                        done[h_] += 1
                        nc.tensor.matmul(
                            sl,
                            lhsT=cur[k],
                            rhs=w_sb[k][:, cc * GW : (cc + 1) * GW],
                            start=False,
                            stop=(done[h_] == KC),
                            skip_group_check=True,
                        )
                    # eltwise; psum layout [i0 f0 o0 g0 i1 f1 o1 g1]
                    ps3 = ps.rearrange("p (c x) -> p c x", c=2)
                    sig = work.tile([B, 2, 384], BF16, tag="sig", name="sig")
                    nc.scalar.activation(
                        sig, ps3[:, :, 0:384], mybir.ActivationFunctionType.Sigmoid
                    )
                    tg = work.tile([B, 2, 128], BF16, tag="tg", name="tg")
                    nc.scalar.activation(
                        tg, ps3[:, :, 384:512], mybir.ActivationFunctionType.Tanh
                    )
                    sig_i = sig[:, :, 0:128]
                    sig_f = sig[:, :, 128:256]
                    sig_o = sig[:, :, 256:384]
                    c3 = c_sb[:, p * 256 : (p + 1) * 256].rearrange(
                        "p (c x) -> p c x", c=2
                    )
                    t1 = work.tile([B, 2, 128], F32, tag="t1", name="t1")
                    nc.vector.tensor_mul(t1, sig_f, c3)
                    t2 = work.tile([B, 2, 128], BF16, tag="t2", name="t2")
                    nc.vector.tensor_mul(t2, sig_i, tg)
                    nc.vector.tensor_add(c3, t1, t2)
                    tanc = work.tile([B, 2, 128], BF16, tag="tanc", name="tanc")
                    nc.scalar.activation(
                        tanc, c3, mybir.ActivationFunctionType.Tanh
                    )
                    hbf = work.tile([B, 2, 128], BF16, tag="hbf", name="hbf")
                    nc.vector.tensor_mul(hbf, sig_o, tanc)
                    for half in range(2):
                        if p >= int(os.environ.get("PE_T_MIN", "0")):
                            # last pair is on the next step's critical path:
                            # PE transpose (~0.4us) beats the DMA xbar (~1.3us)
                            pt = ptp.tile([128, B], BF16, tag="pt", name="pt")
                            nc.tensor.transpose(pt, hbf[:, half, :], ident)
                            nc.vector.tensor_copy(nxt[2 * p + half], pt)
                        else:
                            nc.sync.dma_start_transpose(
                                nxt[2 * p + half], hbf[:, half, :]
                            )

                for cc in range(2 * n_pairs, NCH):  # tail chunks, single width
                    ps1 = psum.tile([B, GW], F32, tag="gates", name=f"ps1_{cc}")
                    cols1 = slice(cc * GW, (cc + 1) * GW)
                    nc.scalar.activation(
                        ps1, wb[:, cols1],
                        mybir.ActivationFunctionType.Copy, scale=xc,
                    )
                    nc.vector.tensor_add(ps1, ps1, bb[:, cols1])
                    for k in range(KC):
                        nc.tensor.matmul(
                            ps1, lhsT=cur[k],
                            rhs=w_sb[k][:, cc * GW : (cc + 1) * GW],
                            start=False, stop=(k == KC - 1),
                            skip_group_check=True,
                        )
                    sig1 = work.tile([B, 384], BF16, tag="sig1", name="sig1")
                    nc.scalar.activation(
                        sig1, ps1[:, 0:384], mybir.ActivationFunctionType.Sigmoid
                    )
                    tg1 = work.tile([B, 128], BF16, tag="tg1", name="tg1")
                    nc.scalar.activation(
                        tg1, ps1[:, 384:512], mybir.ActivationFunctionType.Tanh
                    )
                    c1 = c_sb[:, cc * 128 : (cc + 1) * 128]
                    t1s = work.tile([B, 128], F32, tag="t1s", name="t1s")
                    nc.vector.tensor_mul(t1s, sig1[:, 128:256], c1)
                    t2s = work.tile([B, 128], BF16, tag="t2s", name="t2s")
                    nc.vector.tensor_mul(t2s, sig1[:, 0:128], tg1)
                    nc.vector.tensor_add(c1, t1s, t2s)
                    tanc1 = work.tile([B, 128], BF16, tag="tanc1", name="tanc1")
                    nc.scalar.activation(
                        tanc1, c1, mybir.ActivationFunctionType.Tanh
                    )
                    hbf1 = work.tile([B, 128], BF16, tag="hbf1", name="hbf1")
                    nc.vector.tensor_mul(hbf1, sig1[:, 256:384], tanc1)
                    pt1 = ptp.tile([128, B], BF16, tag="pt", name="pt1")
                    nc.tensor.transpose(pt1, hbf1, ident)
                    nc.vector.tensor_copy(nxt[cc], pt1)

            if t_steps == 0:
                pass
            elif t_steps <= unroll:
                repeat_u = int(os.environ.get("KERNEL_REPEAT", "1"))

                def unrolled_body():
                    for t in range(t_steps):
                        cur, nxt = (ht_a, ht_b) if t % 2 == 0 else (ht_b, ht_a)
                        step(0, t, cur, nxt)

                if repeat_u == 1:
                    unrolled_body()
                else:
                    with tc.For_i(0, repeat_u, 1):
                        unrolled_body()
            else:
                hints = tuple(mybir.ALL_ENGINES) if os.environ.get("HINTS", "0") == "1" else ()
                repeat = int(os.environ.get("KERNEL_REPEAT", "1"))

                def inner_loop():
                    with tc.For_i(0, t_steps, unroll, hint_engines=hints) as iv:
                        for j in range(unroll):
                            cur, nxt = (ht_a, ht_b) if j % 2 == 0 else (ht_b, ht_a)
                            step(iv, j, cur, nxt)

                if repeat == 1:
                    inner_loop()
                else:  # timing amplification only: state re-evolves from t=0 xs
                    with tc.For_i(0, repeat, 1):
                        inner_loop()

            ht_fin = ht_a if t_steps % 2 == 0 else ht_b

            # final FC: out = h_T @ fc_W.T + fc_b
            fc_ps = psum.tile([B, 2 * GW], F32, tag="gates", name="fcps")[:, 0:O]
            nc.tensor.matmul(
                fc_ps, lhsT=ones_t, rhs=fcb_t, start=True, stop=False
            )
            for k in range(KC):
                nc.tensor.matmul(
                    fc_ps,
                    lhsT=ht_fin[k],
                    rhs=fcw[:, k * 128 : (k + 1) * 128],
                    start=False,
                    stop=(k == KC - 1),
                )
            out_sb = work.tile([B, O], F32, tag="out_sb")
            nc.vector.tensor_copy(out_sb, fc_ps)
            nc.gpsimd.dma_start(out=out_d, in_=out_sb)

            if debug_state:
                for k in range(NCH):
                    nc.gpsimd.dma_start(out=ht_dbg_d[k], in_=ht_fin[k])
                nc.gpsimd.dma_start(out=c_dbg_d, in_=c_sb)

    if split_waits:  # required for walrus codegen; CoreSim chokes on it
        _split_excess_waits(nc)
    return nc


# ------------------------------------------------------------------ host prep
def _prep_inputs(y_hist, W_ih, W_hh, b_ih, b_hh, fc_W, fc_b, h0, c0):
    f32 = np.float32
    bf16 = ml_dtypes.bfloat16
    # per-chunk gate permutation of the 4H rows: [i_c | f_c | o_c | g_c]
    # reference gate order in rows of W_hh is (i, f, g, o) blocks of H
    perm = np.concatenate(
        [
            g * H + c * 128 + np.arange(128)
            for c in range(NCH)
            for g in (0, 1, 3, 2)
        ]
    )
    wt = np.ascontiguousarray(W_hh[perm, :].T).astype(bf16)          # (H, 4H)
    wb = np.broadcast_to(W_ih[:, 0][perm].astype(bf16)[None, :], (128, 4 * H))
    bbv = (b_ih + b_hh)[perm].astype(f32)
    bb = np.broadcast_to(bbv[None, :], (128, 4 * H))
    xct = np.ascontiguousarray(y_hist).astype(f32)                    # (B, T)
    ht0 = np.ascontiguousarray(h0.T).astype(bf16)                     # (H, B)
    fcw = np.ascontiguousarray(fc_W.T).astype(bf16)                  # (H, O)
    # device layout for fcw tile: (128, H) with chunk k at cols [128k:128k+128)
    fcw_tile = fcw.reshape(KC, 128, O).transpose(1, 0, 2).reshape(128, H)
    fca = np.stack([fc_b, np.ones(O, f32)]).astype(bf16)              # rhs, ones
    ident = np.eye(128, dtype=f32).astype(bf16)
    return {
        "ident": np.asarray(ident),
        "wt": np.asarray(wt),
        "wb": np.ascontiguousarray(wb),
        "bb": np.ascontiguousarray(bb),
        "xct": xct,
        "ht0": np.asarray(ht0),
        "c0": c0.astype(f32),
        "fcw": np.asarray(fcw_tile),
        "fca": np.asarray(fca),
    }


_CACHE = {}


def _make_runner(nc):
    """Single-core reusable jitted executor (mirrors bass2jax.run_bass_via_pjrt
    but caches the jitted body so repeated kernel() calls skip retracing)."""
    import jax
    from concourse import bass2jax

    bass2jax.install_neuronx_cc_hook()
    partition_name = nc.partition_id_tensor.name if nc.partition_id_tensor else None
    in_names, out_names, out_avals, zero_outs = [], [], [], []
    for alloc in nc.m.functions[0].allocations:
        if not isinstance(alloc, mybir.MemoryLocationSet):
            continue
        name = alloc.memorylocations[0].name
        if alloc.kind == "ExternalInput":
            if name != partition_name:
                in_names.append(name)
        elif alloc.kind == "ExternalOutput":
            shape = tuple(alloc.tensor_shape)
            dtype = mybir.dt.np(alloc.dtype)
            out_names.append(name)
            out_avals.append(jax.core.ShapedArray(shape, dtype))
            zero_outs.append(np.zeros(shape, dtype))
    all_in = list(in_names) + list(out_names)
    if partition_name is not None:
        all_in.append(partition_name)

    def _body(*args):
        operands = list(args)
        if partition_name is not None:
            operands.append(bass2jax.partition_id_tensor())
        return tuple(
            bass2jax._bass_exec_p.bind(
                *operands,
                out_avals=tuple(out_avals),
                in_names=tuple(all_in),
                out_names=tuple(out_names),
                lowering_input_output_aliases=(),
                sim_require_finite=True,
                sim_require_nnan=True,
                nc=nc,
            )
        )

    f = jax.jit(_body, keep_unused=True)
    return f, in_names, out_names, zero_outs


def kernel(y_hist, W_ih, W_hh, b_ih, b_hh, fc_W, fc_b, h0, c0, **kw):
    dev_in = _prep_inputs(
        np.asarray(y_hist, np.float32),
        np.asarray(W_ih, np.float32),
        np.asarray(W_hh, np.float32),
        np.asarray(b_ih, np.float32),
        np.asarray(b_hh, np.float32),
        np.asarray(fc_W, np.float32),
        np.asarray(fc_b, np.float32),
        np.asarray(h0, np.float32),
        np.asarray(c0, np.float32),
    )
    if _N_CORES != 1:
        if "nc" not in _CACHE:
            _CACHE["nc"] = build_program()
        res = run_bass_kernel_spmd(
            _CACHE["nc"],
            [dict(dev_in) for _ in range(_N_CORES)],
            core_ids=list(range(_N_CORES)),
        )
        return np.asarray(res.results[0]["out"], np.float32)
    if "runner" not in _CACHE:
        nc = build_program()
        _CACHE["runner"] = _make_runner(nc)
    f, in_names, out_names, zero_outs = _CACHE["runner"]
    args = [np.asarray(dev_in[n]) for n in in_names] + zero_outs
    outs = f(*args)
    res = {n: np.asarray(outs[i]) for i, n in enumerate(out_names)}
    return np.asarray(res["out"], np.float32)

